# revision 14
# baseline (speedup 1.0000x reference)
"""Trainium2 Bass kernel for nn_Differ (pairwise mu/Sigma differences).

Full-input contract: kernel(mu, Sigma) -> (mu_d, sig_d), each [N*N] f32.

  off-diag (j != k): mu_d[j,k] = mu[j] - mu[k]
                     sig_d[j,k] = S[j,j] + S[k,k] - 2*S[j,k]
  diag     (j == k): mu_d[j,j] = -mu[j]
                     sig_d[j,j] = S[j,j]

Sharding: the j (row) axis of the N x N pairwise grid is split into 8
contiguous blocks of 512 rows, one per NeuronCore.  Each core reads its
512 Sigma rows plus diag(Sigma) and mu, and writes its [512, 4096] block
of both outputs.  The N diagonal elements are overwritten on the host
during unsharding (the device formula gives exactly 0.0 there), which
keeps the SPMD program identical across cores.

Measured design notes:
  - Traffic floor: 8.4 MiB Sigma in + 16.8 MiB out per core; the 16
    SDMA engines pool ~425 GB/s, so the kernel is a streaming problem:
    ~59us of wire that must start early and never trough.  Store lines
    below 8 KiB measurably degrade the pool rate; everything here keeps
    >=8 KiB partition lines.
  - DMA instruction count is kept low (15) on purpose: runs with >=22
    dma_starts repeatably develop a degraded SDMA engine (the dynamic
    queues share one event engine) that drags the tail by ~10us.
    Sigma arrives as ONE rearranged DMA ((t p) c -> p (t c)); sig is
    computed IN-PLACE into the sigma buffer (S is dead after the fused
    op) and ships from there, so stores read single-use tiles and no
    compute op ever blocks on a store-completion round trip.
  - Row vectors (d, -mu) are replicated across partitions by the idle
    TensorEngine as w[1,128]^T @ x (bitwise-exact for fp32 on HW; the
    -ones weight makes mu_ps hold -mu_k so the mu pass is a single
    add).  2 x [128, 4096] f32 broadcasts would be 2x PSUM capacity,
    so the column axis runs in two 2048-wide phases; the mu pass is
    split Vector/Scalar (tensor_scalar_add for tiles 0-1, ACT for 2-3)
    so the phase-A PSUM readers finish early and phase B is not gated
    late.  mu stages full-width (4 stores right as the loads wind
    down); sig halves ship per phase behind them.
"""

import numpy as np

N = 4096
NCORES = 8
RPC = N // NCORES  # 512 rows per core
P = 128            # SBUF partitions
TILES = RPC // P   # 4 row-tiles per core
BANK = 512         # fp32 elements per PSUM bank (matmul N limit)
HALF = N // 2      # column-phase width (2 PSUM residents of 4 banks)

_PROGRAM = None


def _build_program():
    import concourse.bacc as bacc
    import concourse.mybir as mybir
    import concourse.tile as tile
    from concourse.bass import get_trn_type

    f32 = mybir.dt.float32
    ident = mybir.ActivationFunctionType.Identity

    # Bacc (not raw Bass): its generate_event_semaphores pass splits
    # multi-semaphore waits, which TRN2 engines cannot encode (walrus
    # rejects >1 sync wait per instruction).
    nc = bacc.Bacc(
        get_trn_type() or "TRN2",
        target_bir_lowering=False,
        debug=False,
        num_devices=NCORES,
    )
    sigma = nc.declare_dram_parameter("sigma_rows", [RPC, N], f32, isOutput=False)
    # xsvec = [diag(Sigma) || ones(128) || -ones(128) || mu]
    xsvec = nc.declare_dram_parameter("xsvec", [1, 2 * N + 2 * P], f32, isOutput=False)
    # cols[r, t] = d[j0 + t*128 + r], cols[r, TILES+t] = mu[j0 + t*128 + r]
    cols = nc.declare_dram_parameter("cols", [P, 2 * TILES], f32, isOutput=False)
    mu_out = nc.declare_dram_parameter("mu_out", [RPC, N], f32, isOutput=True)
    sig_out = nc.declare_dram_parameter("sig_out", [RPC, N], f32, isOutput=True)

    with tile.TileContext(nc) as tc:
        with (
            tc.tile_pool(name="const", bufs=1) as cpool,
            tc.tile_pool(name="psum", bufs=1, space="PSUM") as ppool,
            tc.tile_pool(name="work", bufs=1) as work,
        ):
            # xs = [d || ones || -ones || mu] in one DMA, first on the
            # sync ring, so the broadcast matmuls start at ~10us.
            xs = cpool.tile([1, 2 * N + 2 * P], f32, tag="xs")
            cols_sb = cpool.tile([P, 2 * TILES], f32, tag="cols")

            nc.sync.dma_start(out=xs[:], in_=xsvec[0:1, :])
            nc.sync.dma_start(out=cols_sb[:], in_=cols[:, :])

            # All 512 sigma rows in ONE DMA: partition p holds rows
            # p, p+128, p+256, p+384 side by side (16 KiB lines).
            s_all = work.tile([P, TILES * N], f32, tag="s")
            nc.sync.dma_start(
                out=s_all[:].rearrange("p (t c) -> p t c", t=TILES),
                in_=sigma[:, :].rearrange("(t p) c -> p t c", p=P),
            )

            ones = xs[0:1, N:N + P]
            nones = xs[0:1, N + P:N + 2 * P]
            MU0 = N + 2 * P  # offset of mu inside xs

            # Both broadcasts live in PSUM, 4 banks each, rebuilt per
            # column phase: w[1,128]^T @ x[1,512] per bank (bitwise
            # exact for fp32, verified on HW).
            mu_ps = ppool.tile([P, HALF], f32, tag="mups")
            d_ps = ppool.tile([P, HALF], f32, tag="dps")

            # Full-width staging for mu: each tile collects both column
            # halves, then goes out as one 2 MiB DMA right as the
            # sigma load winds down.
            m_tiles = [
                work.tile([P, N], f32, tag="m", bufs=TILES, name=f"m{t}")
                for t in range(TILES)
            ]

            for h in range(2):
                c0 = h * HALF
                # mu broadcast first: the mu pass has no sigma dep, so
                # its PSUM readers finish earliest.
                for c in range(HALF // BANK):
                    nc.tensor.matmul(
                        mu_ps[:, c * BANK:(c + 1) * BANK], nones,
                        xs[0:1, MU0 + c0 + c * BANK:MU0 + c0 + (c + 1) * BANK],
                        start=True, stop=True,
                    )
                for c in range(HALF // BANK):
                    nc.tensor.matmul(
                        d_ps[:, c * BANK:(c + 1) * BANK], ones,
                        xs[0:1, c0 + c * BANK:c0 + (c + 1) * BANK],
                        start=True, stop=True,
                    )

                # mu pass: m = (-mu_k) + mu_j; exact negation, so this
                # rounds identically to mu_j - mu_k.  Tiles 2-3 on the
                # Scalar ACT, tiles 0-1 on Vector tensor_scalar_add
                # (GpSimd cannot read PSUM), so the phase-A PSUM readers
                # finish early and phase B is not gated late.
                for t in range(2, TILES):
                    nc.scalar.activation(
                        m_tiles[t][:, c0:c0 + HALF], mu_ps[:], ident,
                        bias=cols_sb[:, TILES + t:TILES + t + 1], scale=1.0,
                    )
                    if h == 1:
                        nc.scalar.dma_start(
                            out=mu_out[t * P:(t + 1) * P, :], in_=m_tiles[t][:]
                        )
                for t in range(2):
                    nc.vector.tensor_scalar_add(
                        m_tiles[t][:, c0:c0 + HALF], mu_ps[:],
                        cols_sb[:, TILES + t:TILES + t + 1],
                    )
                    if h == 1:
                        nc.scalar.dma_start(
                            out=mu_out[t * P:(t + 1) * P, :], in_=m_tiles[t][:]
                        )

                # sig pass: T = d_k + d_j (from PSUM), then one fused
                # DVE op writing sig = (S * -2) + T IN PLACE over S;
                # -2*S is exact and T + (-2S) rounds identically to
                # T - 2S, so this stays bitwise equal to the reference.
                for t in range(TILES):
                    sl = s_all[:, t * N + c0:t * N + c0 + HALF]
                    tt = work.tile([P, HALF], f32, tag="tt", bufs=3, name=f"tt{h}_{t}")
                    nc.scalar.activation(
                        tt[:], d_ps[:], ident,
                        bias=cols_sb[:, t:t + 1], scale=1.0,
                    )
                    nc.vector.scalar_tensor_tensor(
                        sl, sl, -2.0, tt[:],
                        op0=mybir.AluOpType.mult, op1=mybir.AluOpType.add,
                    )
                    # sig halves ride the sync ring, which is done
                    # loading by the time these are ready.
                    nc.sync.dma_start(
                        out=sig_out[t * P:(t + 1) * P, c0:c0 + HALF], in_=sl
                    )

    return nc


def _get_program():
    global _PROGRAM
    if _PROGRAM is None:
        nc = _build_program()
        # Bacc defers register allocation / wait splitting to finalize();
        # the axon PJRT path serializes the module as-is, so run it here.
        nc.finalize()
        _PROGRAM = nc
    return _PROGRAM


def _make_in_maps(mu, Sigma, d):
    xsvec = np.concatenate(
        [d, np.ones(P, np.float32), -np.ones(P, np.float32), mu]
    ).reshape(1, 2 * N + 2 * P)
    in_maps = []
    for c in range(NCORES):
        j0 = c * RPC
        cols = np.concatenate(
            [
                d[j0:j0 + RPC].reshape(TILES, P).T,
                mu[j0:j0 + RPC].reshape(TILES, P).T,
            ],
            axis=1,
        )
        in_maps.append({
            "sigma_rows": np.ascontiguousarray(Sigma[j0:j0 + RPC]),
            "xsvec": xsvec,
            "cols": np.ascontiguousarray(cols),
        })
    return in_maps


def _assemble(per_core_results, mu, d):
    mu_full = np.concatenate(
        [per_core_results[c]["mu_out"] for c in range(NCORES)], axis=0
    )
    sig_full = np.concatenate(
        [per_core_results[c]["sig_out"] for c in range(NCORES)], axis=0
    )
    idx = np.arange(N)
    mu_full[idx, idx] = -mu
    sig_full[idx, idx] = d
    return mu_full.reshape(-1), sig_full.reshape(-1)


def kernel(mu, Sigma, _trace=False):
    from concourse.bass_utils import run_bass_kernel_spmd

    mu = np.ascontiguousarray(np.asarray(mu, dtype=np.float32).reshape(N))
    Sigma = np.ascontiguousarray(np.asarray(Sigma, dtype=np.float32).reshape(N, N))
    d = np.ascontiguousarray(np.diagonal(Sigma)).astype(np.float32)

    nc = _get_program()
    in_maps = _make_in_maps(mu, Sigma, d)
    res = run_bass_kernel_spmd(nc, in_maps, list(range(NCORES)), trace=_trace)
    out = _assemble(res.results, mu, d)
    if _trace:
        return out, res
    return out
